# revision 10
# baseline (speedup 1.0000x reference)
"""3-layer GCN encoder on 8 Trainium2 NeuronCores (Bass/Tile).

Strategy (1D node partitioning):
  - dst nodes sharded contiguously across 8 cores (12500/core).
  - Per layer: each core computes h = (x @ W) * out_norm for its shard,
    AllGather -> full fp16 node-feature table in DRAM.
  - Edge aggregation: edges bucketed by (dst tile, src bucket), messages
    gathered from the table via gpsimd.dma_gather (int16 indices, 4 source
    buckets of <=32K rows each), then segment-summed into PSUM via
    one-hot S-matrix matmuls (S built on DVE with is_equal vs iota).
  - Epilogue per dst tile: (psum + self_loop) * in_norm + b, relu.

Host-side preprocessing (numpy) builds a uniform SPMD plan: per-(tile,
bucket) segment sizes are the max across cores so all 8 cores execute an
identical instruction stream; per-core index/dst-local arrays are data.
"""

import os
import numpy as np

P = 128
N_CORES = 8
TBL_W = 128  # gather table row width (fp16 -> 256B rows)

_BUILD_CACHE = {}

# --------------------------------------------------------------------------
# Host preprocessing: build the uniform aggregation plan
# --------------------------------------------------------------------------


class Plan:
    pass


def make_plan(N, src, dst, n_cores, bucket_limit=32000, st_tiles=5):
    E = src.shape[0]
    Nc = N // n_cores
    assert Nc * n_cores == N
    n_tiles = -(-Nc // P)
    NB = -(-N // bucket_limit)
    BS = -(-N // NB)

    owner = dst // Nc
    r = dst - owner * Nc
    tl = r // P
    dl = (r % P).astype(np.int32)
    bk = src // BS
    sl = (src - bk * BS).astype(np.int32)

    key = (owner.astype(np.int64) * n_tiles + tl) * NB + bk
    order = np.lexsort((src, key))
    key_s = key[order]
    sl_s = sl[order]
    dl_s = dl[order]
    tl_s = tl[order].astype(np.int32)

    n_groups = n_cores * n_tiles * NB
    counts = np.bincount(key, minlength=n_groups).reshape(n_cores, n_tiles, NB)
    maxc = counts.max(axis=0)  # [n_tiles, NB]

    # supertile tile-ranges
    sts = [(i, min(i + st_tiles, n_tiles)) for i in range(0, n_tiles, st_tiles)]
    n_st = len(sts)

    # per supertile/bucket call layout
    # positions are chunk-aligned per call; segments (t,b) packed unaligned
    st_infos = []
    gpos = 0  # global position counter (multiple of 128 at call boundaries)
    icol = 0  # global idx column counter
    npair = 0  # global pair (S window) counter
    # global per-(t,b): position base
    seg_base = np.zeros((n_tiles, NB), dtype=np.int64)
    for si, (t0, t1) in enumerate(sts):
        info = Plan()
        info.t0, info.t1 = t0, t1
        info.calls = []
        info.gpos0 = gpos
        info.icol0 = icol
        st_chunk0 = gpos // P
        for b in range(NB):
            psum_b = int(maxc[t0:t1, b].sum())
            nch = max(1, -(-psum_b // P))
            off = 0
            for t in range(t0, t1):
                seg_base[t, b] = gpos + off
                off += int(maxc[t, b])
            call = Plan()
            call.b = b
            call.row0 = b * BS
            call.nrows = min(BS, N - b * BS)
            call.cb0 = (gpos // P) - st_chunk0  # chunk offset within supertile buffer
            call.nch = nch
            call.lcol0 = icol - info.icol0
            call.ncols = nch * P // 16
            info.calls.append(call)
            gpos += nch * P
            icol += nch * P // 16
        info.nchunks = (gpos // P) - st_chunk0
        info.st_chunk0 = st_chunk0
        info.idx_cols = icol - info.icol0
        st_infos.append(info)

    gpos_tot = gpos
    idx_cols_tot = icol

    # pair enumeration: per (t,b) -> chunk range [qa, qb) (global chunks)
    pair_list = []  # (gq, t)
    tiles_by_st = [[] for _ in range(n_st)]  # per supertile: list of tile dicts
    maxq = 1
    for si, (t0, t1) in enumerate(sts):
        info = st_infos[si]
        for t in range(t0, t1):
            iseq = []  # (b, pair0, npairs)
            pairs = []  # (q_local, pair)
            for b in range(NB):
                if maxc[t, b] == 0:
                    continue
                p0 = int(seg_base[t, b])
                p1 = p0 + int(maxc[t, b])
                qa = p0 // P
                qb = -(-p1 // P)
                pr0 = npair
                for gq in range(qa, qb):
                    pair_list.append((gq, t))
                    pairs.append((gq - info.st_chunk0, npair))
                    npair += 1
                iseq.append((b, pr0, qb - qa))
                maxq = max(maxq, qb - qa)
            tiles_by_st[si].append({"t": t, "iseq": iseq, "pairs": pairs})

    n_pairs_tot = npair
    max_nchunks = max(i.nchunks for i in st_infos)
    max_idx_cols = max(i.idx_cols for i in st_infos)

    # ---------------- per-core data arrays ----------------
    # rank of each edge within its (c,t,b) group
    grp_starts = np.zeros(n_groups + 1, dtype=np.int64)
    np.cumsum(counts.reshape(-1), out=grp_starts[1:])
    rank = np.arange(E, dtype=np.int64) - grp_starts[key_s]

    # seg_base per (t,b) is core-independent
    seg_base_flat = seg_base.reshape(-1)  # [n_tiles*NB]
    tb_of_key = key_s % (n_tiles * NB)
    gpos_of_edge = seg_base_flat[tb_of_key] + rank
    core_of_edge = key_s // (n_tiles * NB)

    idx_flat = np.zeros((n_cores, gpos_tot), dtype=np.int16)
    tile_of_pos = np.full((n_cores, gpos_tot), -1, dtype=np.int16)
    dl_of_pos = np.full((n_cores, gpos_tot), -1, dtype=np.int16)
    idx_flat[core_of_edge, gpos_of_edge] = sl_s.astype(np.int16)
    tile_of_pos[core_of_edge, gpos_of_edge] = tl_s.astype(np.int16)
    dl_of_pos[core_of_edge, gpos_of_edge] = dl_s.astype(np.int16)

    # wrap indices: position i -> [i % 16, i // 16], replicated to 128 rows
    idxw = idx_flat.reshape(n_cores, -1, 16).transpose(0, 2, 1)  # [M,16,cols]
    idxw = np.tile(idxw, (1, 8, 1)).copy()  # [M,128,cols]

    # dst-local per pair: [M, 128, n_pairs]
    dstl = np.full((n_cores, P, n_pairs_tot), -1.0, dtype=np.float32)
    for pi, (gq, t) in enumerate(pair_list):
        s0 = gq * P
        blk_t = tile_of_pos[:, s0:s0 + P]
        blk_d = dl_of_pos[:, s0:s0 + P]
        dstl[:, :, pi] = np.where(blk_t == t, blk_d, -1).astype(np.float32)

    plan = Plan()
    plan.N, plan.E, plan.Nc = N, E, Nc
    plan.n_tiles, plan.NB, plan.BS = n_tiles, NB, BS
    plan.sts, plan.st_infos = sts, st_infos
    plan.tiles_by_st = tiles_by_st
    plan.n_pairs_tot = n_pairs_tot
    plan.idx_cols_tot = idx_cols_tot
    plan.maxq = maxq
    plan.max_nchunks = max_nchunks
    plan.max_idx_cols = max_idx_cols
    plan.idxw, plan.dstl = idxw, dstl
    plan.gpos_tot = gpos_tot
    return plan


# --------------------------------------------------------------------------
# Device kernel builder
# --------------------------------------------------------------------------


def build_kernel(plan, douts):
    from concourse import bass, bacc, tile, mybir

    f32 = mybir.dt.float32
    f16 = mybir.dt.float16
    i16 = mybir.dt.int16

    N, Nc, n_tiles, NB = plan.N, plan.Nc, plan.n_tiles, plan.NB
    NT128 = n_tiles * P

    nc = bacc.Bacc("TRN2", target_bir_lowering=False, debug=False,
                   num_devices=N_CORES)

    xc_d = nc.dram_tensor("xc", [NT128, P], f32, kind="ExternalInput")
    W_d = [nc.dram_tensor(f"W{l}", [P, douts[l]], f32, kind="ExternalInput")
           for l in range(3)]
    B_d = [nc.dram_tensor(f"B{l}", [P, douts[l]], f32, kind="ExternalInput")
           for l in range(3)]
    onorm_d = nc.dram_tensor("onorm", [P, n_tiles], f32, kind="ExternalInput")
    inorm_d = nc.dram_tensor("inorm", [P, n_tiles], f32, kind="ExternalInput")
    iota_d = nc.dram_tensor("iota", [P, plan.maxq * P], f16, kind="ExternalInput")
    ident_d = nc.dram_tensor("ident", [P, P], f32, kind="ExternalInput")
    idxw_d = nc.dram_tensor("idxw", [P, plan.idx_cols_tot], i16, kind="ExternalInput")
    dstl_d = nc.dram_tensor("dstl", [P, plan.n_pairs_tot], f32, kind="ExternalInput")
    out_d = nc.dram_tensor("out", [Nc, douts[2]], f32, kind="ExternalOutput")

    with tile.TileContext(nc) as tc:
        with tc.tile_pool(name="const", bufs=1) as cp, \
             tc.tile_pool(name="msgp", bufs=2) as mp, \
             tc.tile_pool(name="idxp", bufs=2) as ip, \
             tc.tile_pool(name="sp", bufs=4) as sp, \
             tc.tile_pool(name="ep", bufs=3) as ep, \
             tc.tile_pool(name="xtp", bufs=3) as xtp, \
             tc.tile_pool(name="psA", bufs=2, space="PSUM") as psA, \
             tc.tile_pool(name="psC", bufs=4, space="PSUM") as psC, \
             tc.tile_pool(name="dram", bufs=2, space="DRAM") as dr:

            # resident tiles
            x_sb = cp.tile([P, NT128], f32, tag="x")
            hs_sb = cp.tile([P, NT128], f16, tag="hs")
            W_sb = [cp.tile([P, douts[l]], f32, tag=f"W{l}", name=f"W{l}_sb") for l in range(3)]
            B_sb = [cp.tile([P, douts[l]], f32, tag=f"B{l}", name=f"B{l}_sb") for l in range(3)]
            on_sb = cp.tile([P, n_tiles], f32, tag="on")
            in_sb = cp.tile([P, n_tiles], f32, tag="in")
            iota_sb = cp.tile([P, plan.maxq * P], f16, tag="iota")
            id_sb = cp.tile([P, P], f32, tag="ident")
            dstl_sb = cp.tile([P, plan.n_pairs_tot], f32, tag="dstl")

            for l in range(3):
                nc.sync.dma_start(W_sb[l][:], W_d[l].ap())
                nc.sync.dma_start(B_sb[l][:], B_d[l].ap())
            nc.sync.dma_start(on_sb[:], onorm_d.ap())
            nc.sync.dma_start(in_sb[:], inorm_d.ap())
            nc.sync.dma_start(iota_sb[:], iota_d.ap())
            nc.sync.dma_start(id_sb[:], ident_d.ap())
            nc.sync.dma_start(dstl_sb[:], dstl_d.ap())
            nc.sync.dma_start(
                x_sb[:].rearrange("p (t f) -> p t f", f=P),
                xc_d.ap().rearrange("(t p) f -> p t f", p=P))

            for l in range(3):
                d = douts[l]
                if d < TBL_W:
                    nc.gpsimd.memset(hs_sb[:], 0)
                ag_in = dr.tile([Nc, TBL_W], f16, tag="agin")
                h_full = dr.tile([N, TBL_W], f16, tag="hfull",
                                 addr_space="Shared")

                # ---- phase A: h = (x @ W) * out_norm ----
                for t in range(n_tiles):
                    rows = min(P, Nc - t * P)
                    xt_ps = psA.tile([P, P], f32, tag="tp")
                    nc.tensor.transpose(xt_ps[:], x_sb[:, t * P:(t + 1) * P],
                                        id_sb[:])
                    xT = xtp.tile([P, P], f32, tag="xT")
                    nc.scalar.copy(xT[:], xt_ps[:])
                    h_ps = psA.tile([P, d], f32, tag="hp")
                    nc.tensor.matmul(h_ps[:], lhsT=xT[:], rhs=W_sb[l][:],
                                     start=True, stop=True)
                    nc.scalar.mul(hs_sb[:, t * P:t * P + d], h_ps[:],
                                  on_sb[:, t:t + 1])
                    nc.sync.dma_start(
                        ag_in[t * P:t * P + rows, :],
                        hs_sb[:rows, t * P:(t + 1) * P])

                # ---- phase B: AllGather ----
                nc.gpsimd.collective_compute(
                    "AllGather", bass.mybir.AluOpType.bypass,
                    replica_groups=[list(range(N_CORES))],
                    ins=[ag_in.opt()],
                    outs=[h_full.opt()])

                # ---- phase C: gather + aggregate + epilogue ----
                for si, info in enumerate(plan.st_infos):
                    msg = mp.tile([P, plan.max_nchunks * P], f16, tag="msg")
                    idxt = ip.tile([P, plan.max_idx_cols], i16, tag="idx")
                    nc.sync.dma_start(
                        idxt[:, :info.idx_cols],
                        idxw_d.ap()[:, info.icol0:info.icol0 + info.idx_cols])
                    for call in info.calls:
                        oap = msg[:, call.cb0 * P:(call.cb0 + call.nch) * P]
                        oap = oap.rearrange("p (q e) -> p q e", e=P)
                        nc.gpsimd.dma_gather(
                            out_ap=oap,
                            in_ap=h_full[call.row0:call.row0 + call.nrows, :],
                            idxs_ap=idxt[:, call.lcol0:call.lcol0 + call.ncols],
                            num_idxs=call.nch * P,
                            num_idxs_reg=call.nch * P,
                            elem_size=TBL_W,
                            single_packet=False)

                    for tinfo in plan.tiles_by_st[si]:
                        t = tinfo["t"]
                        pairs = tinfo["pairs"]
                        rows = min(P, Nc - t * P)
                        # build S per (t,b), then matmul-accumulate
                        pair_home = {}
                        seq_build = os.environ.get("GCN_SEQ_BUILD", "0") == "1"
                        for (b, pr0, npr) in tinfo["iseq"]:
                            S = sp.tile([P, plan.maxq * P], f16, tag="S")
                            if seq_build:
                                for k in range(npr):
                                    nc.vector.tensor_scalar(
                                        S[:, k * P:(k + 1) * P],
                                        iota_sb[:, :P],
                                        dstl_sb[:, pr0 + k:pr0 + k + 1],
                                        None, mybir.AluOpType.is_equal)
                            else:
                                nc.vector.tensor_tensor(
                                    S[:, :npr * P].rearrange(
                                        "p (q e) -> p q e", e=P),
                                    iota_sb[:, :npr * P].rearrange(
                                        "p (q e) -> p q e", e=P),
                                    dstl_sb[:, pr0:pr0 + npr].broadcast_to(
                                        (P, npr, P)),
                                    mybir.AluOpType.is_equal)
                            for k in range(npr):
                                pair_home[pr0 + k] = (S, k)
                        ps = None
                        if pairs:
                            ps = psC.tile([P, d], f32, tag="pc")
                            for i, (q_local, pr) in enumerate(pairs):
                                S, k = pair_home[pr]
                                nc.tensor.matmul(
                                    ps[:],
                                    lhsT=S[:, k * P:(k + 1) * P],
                                    rhs=msg[:, q_local * P:q_local * P + d],
                                    start=(i == 0),
                                    stop=(i == len(pairs) - 1))
                        # epilogue
                        hsf = ep.tile([P, d], f32, tag="hsf")
                        nc.scalar.copy(hsf[:], hs_sb[:, t * P:t * P + d])
                        t1 = ep.tile([P, d], f32, tag="t1")
                        if ps is not None:
                            nc.vector.tensor_tensor(t1[:], ps[:], hsf[:], mybir.AluOpType.add)
                        else:
                            t1 = hsf
                        t2 = ep.tile([P, d], f32, tag="t2")
                        nc.vector.tensor_scalar(t2[:], t1[:], in_sb[:, t:t + 1],
                                                None, mybir.AluOpType.mult)
                        t3 = ep.tile([P, d], f32, tag="t3")
                        nc.vector.tensor_tensor(t3[:], t2[:], B_sb[l][:], mybir.AluOpType.add)
                        if l < 2:
                            nc.vector.tensor_scalar(
                                x_sb[:, t * P:t * P + d], t3[:], 0.0, None,
                                mybir.AluOpType.max)
                        else:
                            nc.sync.dma_start(out_d.ap()[t * P:t * P + rows, :],
                                              t3[:rows, :])

    nc.compile()
    return nc


# --------------------------------------------------------------------------
# Entry point
# --------------------------------------------------------------------------

LAST_EXEC_NS = None


def kernel(feat, src, dst, W1, b1, W2, b2, W3, b3):
    global LAST_EXEC_NS
    from concourse.bass_utils import run_bass_kernel_spmd

    feat = np.asarray(feat, dtype=np.float32)
    src = np.asarray(src, dtype=np.int32)
    dst = np.asarray(dst, dtype=np.int32)
    Ws = [np.asarray(w, np.float32) for w in (W1, W2, W3)]
    bs = [np.asarray(b, np.float32) for b in (b1, b2, b3)]

    N, F = feat.shape
    douts = [w.shape[1] for w in Ws]

    bucket_limit = int(os.environ.get("GCN_BUCKET_LIMIT", "32000"))
    st_tiles = int(os.environ.get("GCN_ST_TILES", "5"))

    import hashlib
    h = hashlib.sha1()
    h.update(src.tobytes())
    h.update(dst.tobytes())
    key = (N, F, tuple(douts), h.hexdigest(), bucket_limit, st_tiles)

    if key in _BUILD_CACHE:
        nc, plan = _BUILD_CACHE[key]
    else:
        plan = make_plan(N, src, dst, N_CORES, bucket_limit, st_tiles)
        nc = build_kernel(plan, douts)
        _BUILD_CACHE.clear()
        _BUILD_CACHE[key] = (nc, plan)

    Nc, n_tiles = plan.Nc, plan.n_tiles

    # norms (degrees include self-loops)
    deg_out = np.bincount(src, minlength=N).astype(np.float32) + 1.0
    deg_in = np.bincount(dst, minlength=N).astype(np.float32) + 1.0
    out_norm = (1.0 / np.sqrt(deg_out)).astype(np.float32)
    in_norm = (1.0 / np.sqrt(deg_in)).astype(np.float32)

    NT128 = n_tiles * P
    iota = np.tile(np.arange(P, dtype=np.float16), plan.maxq)
    iota = np.broadcast_to(iota, (P, plan.maxq * P)).copy()
    ident = np.eye(P, dtype=np.float32)

    in_maps = []
    for c in range(N_CORES):
        xc = np.zeros((NT128, P), np.float32)
        xc[:Nc] = feat[c * Nc:(c + 1) * Nc]
        onorm = np.zeros((P, n_tiles), np.float32)
        inorm = np.zeros((P, n_tiles), np.float32)
        on_pad = np.zeros(NT128, np.float32)
        on_pad[:Nc] = out_norm[c * Nc:(c + 1) * Nc]
        in_pad = np.zeros(NT128, np.float32)
        in_pad[:Nc] = in_norm[c * Nc:(c + 1) * Nc]
        onorm[:, :] = on_pad.reshape(n_tiles, P).T
        inorm[:, :] = in_pad.reshape(n_tiles, P).T
        m = {
            "xc": xc,
            "onorm": onorm,
            "inorm": inorm,
            "iota": iota,
            "ident": ident,
            "idxw": plan.idxw[c],
            "dstl": plan.dstl[c],
        }
        for l in range(3):
            m[f"W{l}"] = Ws[l]
            m[f"B{l}"] = np.broadcast_to(bs[l], (P, douts[l])).copy()
        in_maps.append(m)

    trace = os.environ.get("GCN_TRACE", "0") == "1"
    res = run_bass_kernel_spmd(nc, in_maps, core_ids=list(range(N_CORES)),
                               trace=trace)
    LAST_EXEC_NS = res.exec_time_ns
    out = np.concatenate([res.results[c]["out"] for c in range(N_CORES)],
                         axis=0)
    return out[:N].astype(np.float32)


# revision 11
# speedup vs baseline: 1.8659x; 1.8659x over previous
"""3-layer GCN encoder on 8 Trainium2 NeuronCores (Bass/Tile).

Strategy (1D node partitioning):
  - dst nodes sharded contiguously across 8 cores (12500/core).
  - Per layer: each core computes h = (x @ W) * out_norm for its shard,
    AllGather -> full fp16 node-feature table in DRAM.
  - Edge aggregation: edges bucketed by (dst tile, src bucket), messages
    gathered from the table via gpsimd.dma_gather (int16 indices, 4 source
    buckets of <=32K rows each), then segment-summed into PSUM via
    one-hot S-matrix matmuls (S built on DVE with is_equal vs iota).
  - Epilogue per dst tile: (psum + self_loop) * in_norm + b, relu.

Host-side preprocessing (numpy) builds a uniform SPMD plan: per-(tile,
bucket) segment sizes are the max across cores so all 8 cores execute an
identical instruction stream; per-core index/dst-local arrays are data.
"""

import os
import numpy as np

P = 128
N_CORES = 8
TBL_W = 128  # gather table row width (fp16 -> 256B rows)

_BUILD_CACHE = {}

# --------------------------------------------------------------------------
# Host preprocessing: build the uniform aggregation plan
# --------------------------------------------------------------------------


class Plan:
    pass


def make_plan(N, src, dst, n_cores, bucket_limit=32000, st_tiles=5):
    E = src.shape[0]
    Nc = N // n_cores
    assert Nc * n_cores == N
    n_tiles = -(-Nc // P)
    NB = -(-N // bucket_limit)
    BS = -(-N // NB)

    owner = dst // Nc
    r = dst - owner * Nc
    tl = r // P
    dl = (r % P).astype(np.int32)
    bk = src // BS
    sl = (src - bk * BS).astype(np.int32)

    key = (owner.astype(np.int64) * n_tiles + tl) * NB + bk
    order = np.lexsort((src, key))
    key_s = key[order]
    sl_s = sl[order]
    dl_s = dl[order]
    tl_s = tl[order].astype(np.int32)

    n_groups = n_cores * n_tiles * NB
    counts = np.bincount(key, minlength=n_groups).reshape(n_cores, n_tiles, NB)
    maxc = counts.max(axis=0)  # [n_tiles, NB]

    # supertile tile-ranges
    sts = [(i, min(i + st_tiles, n_tiles)) for i in range(0, n_tiles, st_tiles)]
    n_st = len(sts)

    # per supertile/bucket call layout
    # positions are chunk-aligned per call; segments (t,b) packed unaligned
    st_infos = []
    gpos = 0  # global position counter (multiple of 128 at call boundaries)
    icol = 0  # global idx column counter
    npair = 0  # global pair (S window) counter
    # global per-(t,b): position base
    seg_base = np.zeros((n_tiles, NB), dtype=np.int64)
    for si, (t0, t1) in enumerate(sts):
        info = Plan()
        info.t0, info.t1 = t0, t1
        info.calls = []
        info.gpos0 = gpos
        info.icol0 = icol
        st_chunk0 = gpos // P
        for b in range(NB):
            psum_b = int(maxc[t0:t1, b].sum())
            nch = max(1, -(-psum_b // P))
            off = 0
            for t in range(t0, t1):
                seg_base[t, b] = gpos + off
                off += int(maxc[t, b])
            call = Plan()
            call.b = b
            call.row0 = b * BS
            call.nrows = min(BS, N - b * BS)
            call.cb0 = (gpos // P) - st_chunk0  # chunk offset within supertile buffer
            call.nch = nch
            call.lcol0 = icol - info.icol0
            call.ncols = nch * P // 16
            info.calls.append(call)
            gpos += nch * P
            icol += nch * P // 16
        info.nchunks = (gpos // P) - st_chunk0
        info.st_chunk0 = st_chunk0
        info.idx_cols = icol - info.icol0
        st_infos.append(info)

    gpos_tot = gpos
    idx_cols_tot = icol

    # pair enumeration: per (t,b) -> chunk range [qa, qb) (global chunks)
    pair_list = []  # (gq, t)
    tiles_by_st = [[] for _ in range(n_st)]  # per supertile: list of tile dicts
    maxq = 1
    for si, (t0, t1) in enumerate(sts):
        info = st_infos[si]
        for t in range(t0, t1):
            iseq = []  # (b, pair0, npairs)
            pairs = []  # (q_local, pair)
            for b in range(NB):
                if maxc[t, b] == 0:
                    continue
                p0 = int(seg_base[t, b])
                p1 = p0 + int(maxc[t, b])
                qa = p0 // P
                qb = -(-p1 // P)
                pr0 = npair
                for gq in range(qa, qb):
                    pair_list.append((gq, t))
                    pairs.append((gq - info.st_chunk0, npair))
                    npair += 1
                iseq.append((b, pr0, qb - qa))
                maxq = max(maxq, qb - qa)
            tiles_by_st[si].append({"t": t, "iseq": iseq, "pairs": pairs})

    n_pairs_tot = npair
    max_nchunks = max(i.nchunks for i in st_infos)
    max_idx_cols = max(i.idx_cols for i in st_infos)

    # ---------------- per-core data arrays ----------------
    # rank of each edge within its (c,t,b) group
    grp_starts = np.zeros(n_groups + 1, dtype=np.int64)
    np.cumsum(counts.reshape(-1), out=grp_starts[1:])
    rank = np.arange(E, dtype=np.int64) - grp_starts[key_s]

    # seg_base per (t,b) is core-independent
    seg_base_flat = seg_base.reshape(-1)  # [n_tiles*NB]
    tb_of_key = key_s % (n_tiles * NB)
    gpos_of_edge = seg_base_flat[tb_of_key] + rank
    core_of_edge = key_s // (n_tiles * NB)

    idx_flat = np.zeros((n_cores, gpos_tot), dtype=np.int16)
    tile_of_pos = np.full((n_cores, gpos_tot), -1, dtype=np.int16)
    dl_of_pos = np.full((n_cores, gpos_tot), -1, dtype=np.int16)
    idx_flat[core_of_edge, gpos_of_edge] = sl_s.astype(np.int16)
    tile_of_pos[core_of_edge, gpos_of_edge] = tl_s.astype(np.int16)
    dl_of_pos[core_of_edge, gpos_of_edge] = dl_s.astype(np.int16)

    # wrap indices: position i -> [i % 16, i // 16], replicated to 128 rows
    idxw = idx_flat.reshape(n_cores, -1, 16).transpose(0, 2, 1)  # [M,16,cols]
    idxw = np.tile(idxw, (1, 8, 1)).copy()  # [M,128,cols]

    # dst-local per pair: [M, 128, n_pairs]
    dstl = np.full((n_cores, P, n_pairs_tot), -1.0, dtype=np.float32)
    for pi, (gq, t) in enumerate(pair_list):
        s0 = gq * P
        blk_t = tile_of_pos[:, s0:s0 + P]
        blk_d = dl_of_pos[:, s0:s0 + P]
        dstl[:, :, pi] = np.where(blk_t == t, blk_d, -1).astype(np.float32)

    plan = Plan()
    plan.N, plan.E, plan.Nc = N, E, Nc
    plan.n_tiles, plan.NB, plan.BS = n_tiles, NB, BS
    plan.sts, plan.st_infos = sts, st_infos
    plan.tiles_by_st = tiles_by_st
    plan.n_pairs_tot = n_pairs_tot
    plan.idx_cols_tot = idx_cols_tot
    plan.maxq = maxq
    plan.max_nchunks = max_nchunks
    plan.max_idx_cols = max_idx_cols
    plan.idxw, plan.dstl = idxw, dstl
    plan.gpos_tot = gpos_tot
    return plan


# --------------------------------------------------------------------------
# Device kernel builder
# --------------------------------------------------------------------------


def build_kernel(plan, douts):
    from concourse import bass, bacc, tile, mybir

    f32 = mybir.dt.float32
    f16 = mybir.dt.float16
    i16 = mybir.dt.int16

    N, Nc, n_tiles, NB = plan.N, plan.Nc, plan.n_tiles, plan.NB
    NT128 = n_tiles * P

    nc = bacc.Bacc("TRN2", target_bir_lowering=False, debug=False,
                   num_devices=N_CORES, num_swdge_queues=4)

    xc_d = nc.dram_tensor("xc", [NT128, P], f32, kind="ExternalInput")
    W_d = [nc.dram_tensor(f"W{l}", [P, douts[l]], f32, kind="ExternalInput")
           for l in range(3)]
    B_d = [nc.dram_tensor(f"B{l}", [P, douts[l]], f32, kind="ExternalInput")
           for l in range(3)]
    onorm_d = nc.dram_tensor("onorm", [P, n_tiles], f32, kind="ExternalInput")
    inorm_d = nc.dram_tensor("inorm", [P, n_tiles], f32, kind="ExternalInput")
    iota_d = nc.dram_tensor("iota", [P, plan.maxq * P], f16, kind="ExternalInput")
    ident_d = nc.dram_tensor("ident", [P, P], f32, kind="ExternalInput")
    idxw_d = nc.dram_tensor("idxw", [P, plan.idx_cols_tot], i16, kind="ExternalInput")
    dstl_d = nc.dram_tensor("dstl", [P, plan.n_pairs_tot], f32, kind="ExternalInput")
    out_d = nc.dram_tensor("out", [Nc, douts[2]], f32, kind="ExternalOutput")

    with tile.TileContext(nc) as tc:
        with tc.tile_pool(name="const", bufs=1) as cp, \
             tc.tile_pool(name="msgp", bufs=2) as mp, \
             tc.tile_pool(name="idxp", bufs=2) as ip, \
             tc.tile_pool(name="sp", bufs=4) as sp, \
             tc.tile_pool(name="ep", bufs=3) as ep, \
             tc.tile_pool(name="xtp", bufs=3) as xtp, \
             tc.tile_pool(name="psA", bufs=2, space="PSUM") as psA, \
             tc.tile_pool(name="psC", bufs=4, space="PSUM") as psC, \
             tc.tile_pool(name="dram", bufs=2, space="DRAM") as dr:

            # resident tiles
            x_sb = cp.tile([P, NT128], f32, tag="x")
            hs_sb = cp.tile([P, NT128], f16, tag="hs")
            W_sb = [cp.tile([P, douts[l]], f32, tag=f"W{l}", name=f"W{l}_sb") for l in range(3)]
            B_sb = [cp.tile([P, douts[l]], f32, tag=f"B{l}", name=f"B{l}_sb") for l in range(3)]
            on_sb = cp.tile([P, n_tiles], f32, tag="on")
            in_sb = cp.tile([P, n_tiles], f32, tag="in")
            iota_sb = cp.tile([P, plan.maxq * P], f16, tag="iota")
            id_sb = cp.tile([P, P], f32, tag="ident")
            dstl_sb = cp.tile([P, plan.n_pairs_tot], f32, tag="dstl")

            for l in range(3):
                nc.sync.dma_start(W_sb[l][:], W_d[l].ap())
                nc.sync.dma_start(B_sb[l][:], B_d[l].ap())
            nc.sync.dma_start(on_sb[:], onorm_d.ap())
            nc.sync.dma_start(in_sb[:], inorm_d.ap())
            nc.sync.dma_start(iota_sb[:], iota_d.ap())
            nc.sync.dma_start(id_sb[:], ident_d.ap())
            nc.sync.dma_start(dstl_sb[:], dstl_d.ap())
            nc.sync.dma_start(
                x_sb[:].rearrange("p (t f) -> p t f", f=P),
                xc_d.ap().rearrange("(t p) f -> p t f", p=P))

            for l in range(3):
                d = douts[l]
                if d < TBL_W:
                    nc.gpsimd.memset(hs_sb[:], 0)
                ag_in = dr.tile([Nc, TBL_W], f16, tag="agin")
                h_full = dr.tile([N, TBL_W], f16, tag="hfull",
                                 addr_space="Shared")

                # ---- phase A: h = (x @ W) * out_norm ----
                for t in range(n_tiles):
                    rows = min(P, Nc - t * P)
                    xt_ps = psA.tile([P, P], f32, tag="tp")
                    nc.tensor.transpose(xt_ps[:], x_sb[:, t * P:(t + 1) * P],
                                        id_sb[:])
                    xT = xtp.tile([P, P], f32, tag="xT")
                    nc.scalar.copy(xT[:], xt_ps[:])
                    h_ps = psA.tile([P, d], f32, tag="hp")
                    nc.tensor.matmul(h_ps[:], lhsT=xT[:], rhs=W_sb[l][:],
                                     start=True, stop=True)
                    nc.scalar.mul(hs_sb[:, t * P:t * P + d], h_ps[:],
                                  on_sb[:, t:t + 1])
                    nc.sync.dma_start(
                        ag_in[t * P:t * P + rows, :],
                        hs_sb[:rows, t * P:(t + 1) * P])

                # ---- phase B: AllGather ----
                nc.gpsimd.collective_compute(
                    "AllGather", bass.mybir.AluOpType.bypass,
                    replica_groups=[list(range(N_CORES))],
                    ins=[ag_in.opt()],
                    outs=[h_full.opt()])

                # ---- phase C: gather + aggregate + epilogue ----
                gq_rr = 0
                for si, info in enumerate(plan.st_infos):
                    msg = mp.tile([P, plan.max_nchunks * P], f16, tag="msg")
                    idxt = ip.tile([P, plan.max_idx_cols], i16, tag="idx")
                    nc.sync.dma_start(
                        idxt[:, :info.idx_cols],
                        idxw_d.ap()[:, info.icol0:info.icol0 + info.idx_cols])
                    for call in info.calls:
                        oap = msg[:, call.cb0 * P:(call.cb0 + call.nch) * P]
                        oap = oap.rearrange("p (q e) -> p q e", e=P)
                        nc.gpsimd.dma_gather(
                            out_ap=oap,
                            in_ap=h_full[call.row0:call.row0 + call.nrows, :],
                            idxs_ap=idxt[:, call.lcol0:call.lcol0 + call.ncols],
                            num_idxs=call.nch * P,
                            num_idxs_reg=call.nch * P,
                            elem_size=TBL_W,
                            single_packet=False,
                            queue_num=gq_rr % 4)
                        gq_rr += 1

                    for tinfo in plan.tiles_by_st[si]:
                        t = tinfo["t"]
                        pairs = tinfo["pairs"]
                        rows = min(P, Nc - t * P)
                        # build S per (t,b), then matmul-accumulate
                        pair_home = {}
                        seq_build = os.environ.get("GCN_SEQ_BUILD", "1") == "1"
                        for (b, pr0, npr) in tinfo["iseq"]:
                            S = sp.tile([P, plan.maxq * P], f16, tag="S")
                            if seq_build:
                                for k in range(npr):
                                    nc.vector.tensor_scalar(
                                        S[:, k * P:(k + 1) * P],
                                        iota_sb[:, :P],
                                        dstl_sb[:, pr0 + k:pr0 + k + 1],
                                        None, mybir.AluOpType.is_equal)
                            else:
                                nc.vector.tensor_tensor(
                                    S[:, :npr * P].rearrange(
                                        "p (q e) -> p q e", e=P),
                                    iota_sb[:, :npr * P].rearrange(
                                        "p (q e) -> p q e", e=P),
                                    dstl_sb[:, pr0:pr0 + npr].broadcast_to(
                                        (P, npr, P)),
                                    mybir.AluOpType.is_equal)
                            for k in range(npr):
                                pair_home[pr0 + k] = (S, k)
                        ps = None
                        if pairs:
                            ps = psC.tile([P, d], f32, tag="pc")
                            for i, (q_local, pr) in enumerate(pairs):
                                S, k = pair_home[pr]
                                nc.tensor.matmul(
                                    ps[:],
                                    lhsT=S[:, k * P:(k + 1) * P],
                                    rhs=msg[:, q_local * P:q_local * P + d],
                                    start=(i == 0),
                                    stop=(i == len(pairs) - 1))
                        # epilogue
                        hsf = ep.tile([P, d], f32, tag="hsf")
                        nc.scalar.copy(hsf[:], hs_sb[:, t * P:t * P + d])
                        t1 = ep.tile([P, d], f32, tag="t1")
                        if ps is not None:
                            nc.vector.tensor_tensor(t1[:], ps[:], hsf[:], mybir.AluOpType.add)
                        else:
                            t1 = hsf
                        t2 = ep.tile([P, d], f32, tag="t2")
                        nc.vector.tensor_scalar(t2[:], t1[:], in_sb[:, t:t + 1],
                                                None, mybir.AluOpType.mult)
                        t3 = ep.tile([P, d], f32, tag="t3")
                        nc.vector.tensor_tensor(t3[:], t2[:], B_sb[l][:], mybir.AluOpType.add)
                        if l < 2:
                            nc.vector.tensor_scalar(
                                x_sb[:, t * P:t * P + d], t3[:], 0.0, None,
                                mybir.AluOpType.max)
                        else:
                            nc.sync.dma_start(out_d.ap()[t * P:t * P + rows, :],
                                              t3[:rows, :])

    nc.compile()
    return nc


# --------------------------------------------------------------------------
# Entry point
# --------------------------------------------------------------------------

LAST_EXEC_NS = None


def kernel(feat, src, dst, W1, b1, W2, b2, W3, b3):
    global LAST_EXEC_NS
    from concourse.bass_utils import run_bass_kernel_spmd

    feat = np.asarray(feat, dtype=np.float32)
    src = np.asarray(src, dtype=np.int32)
    dst = np.asarray(dst, dtype=np.int32)
    Ws = [np.asarray(w, np.float32) for w in (W1, W2, W3)]
    bs = [np.asarray(b, np.float32) for b in (b1, b2, b3)]

    N, F = feat.shape
    douts = [w.shape[1] for w in Ws]

    bucket_limit = int(os.environ.get("GCN_BUCKET_LIMIT", "32000"))
    st_tiles = int(os.environ.get("GCN_ST_TILES", "5"))

    import hashlib
    h = hashlib.sha1()
    h.update(src.tobytes())
    h.update(dst.tobytes())
    key = (N, F, tuple(douts), h.hexdigest(), bucket_limit, st_tiles)

    if key in _BUILD_CACHE:
        nc, plan = _BUILD_CACHE[key]
    else:
        plan = make_plan(N, src, dst, N_CORES, bucket_limit, st_tiles)
        nc = build_kernel(plan, douts)
        _BUILD_CACHE.clear()
        _BUILD_CACHE[key] = (nc, plan)

    Nc, n_tiles = plan.Nc, plan.n_tiles

    # norms (degrees include self-loops)
    deg_out = np.bincount(src, minlength=N).astype(np.float32) + 1.0
    deg_in = np.bincount(dst, minlength=N).astype(np.float32) + 1.0
    out_norm = (1.0 / np.sqrt(deg_out)).astype(np.float32)
    in_norm = (1.0 / np.sqrt(deg_in)).astype(np.float32)

    NT128 = n_tiles * P
    iota = np.tile(np.arange(P, dtype=np.float16), plan.maxq)
    iota = np.broadcast_to(iota, (P, plan.maxq * P)).copy()
    ident = np.eye(P, dtype=np.float32)

    in_maps = []
    for c in range(N_CORES):
        xc = np.zeros((NT128, P), np.float32)
        xc[:Nc] = feat[c * Nc:(c + 1) * Nc]
        onorm = np.zeros((P, n_tiles), np.float32)
        inorm = np.zeros((P, n_tiles), np.float32)
        on_pad = np.zeros(NT128, np.float32)
        on_pad[:Nc] = out_norm[c * Nc:(c + 1) * Nc]
        in_pad = np.zeros(NT128, np.float32)
        in_pad[:Nc] = in_norm[c * Nc:(c + 1) * Nc]
        onorm[:, :] = on_pad.reshape(n_tiles, P).T
        inorm[:, :] = in_pad.reshape(n_tiles, P).T
        m = {
            "xc": xc,
            "onorm": onorm,
            "inorm": inorm,
            "iota": iota,
            "ident": ident,
            "idxw": plan.idxw[c],
            "dstl": plan.dstl[c],
        }
        for l in range(3):
            m[f"W{l}"] = Ws[l]
            m[f"B{l}"] = np.broadcast_to(bs[l], (P, douts[l])).copy()
        in_maps.append(m)

    trace = os.environ.get("GCN_TRACE", "0") == "1"
    res = run_bass_kernel_spmd(nc, in_maps, core_ids=list(range(N_CORES)),
                               trace=trace)
    LAST_EXEC_NS = res.exec_time_ns
    out = np.concatenate([res.results[c]["out"] for c in range(N_CORES)],
                         axis=0)
    return out[:N].astype(np.float32)
